# revision 1
# baseline (speedup 1.0000x reference)
"""Trainium2 Bass kernel for nn_MultiHeadAttention_65481071395029.

8-core SPMD: core c handles batch b=c//2 and heads h0=(c%2)*8 .. h0+8.
Math per core (S=1024, DK=64, 8 heads):
  q = query @ WqT/8 + bq/8        (transposed layout: [dk, s])
  k = key   @ WkT   + bk
  asprow_h = tanh(aw_h . k_h + bias_m)   with aw = (aspect @ WdT + bd) @ weight_m
  scores = q_h.T k_h (+ ones x asprow via 65-row contraction) + short + maskbias
  out = softmax(scores, axis=-1) = exp(scores)/rowsum (no max-subtract needed:
  unmasked scores are O(10); masked entries sit at ~-60000 and underflow to 0)

Engine plan per (head, qtile) iteration over a [128,1024] score tile:
  PE:  2x QK matmuls (fp16, contraction 65 = dk+aspect row) start PSUM,
       2x identity-inject matmuls accumulate sm = short+maskbias on top
  ACT: one Exp pass PSUM->SBUF fp16 with accumulated rowsum
  DVE: reciprocal + 4x tensor_scalar scale
  DMA: 1MB transfers (4 qtiles per group), all on the sync HWDGE ring.
Projections run as head-pair matmuls [128(2x dk), s] with the bias add fused
into the PSUM evacuation, then are rearranged into per-head [65, s] tiles
(ones/aspect row appended) via on-chip DMA. The remaining projections are
emitted in small chunks between main-loop groups so the PE never diverts
longer than the PSUM buffering can hide from the ACT-paced exp stream.
"""

import numpy as np
from contextlib import ExitStack

B, S, D, H, DK = 4, 1024, 1024, 16, 64
HPC = 8          # heads per core
NPAIR = HPC // 2
QTN = S // 128   # q tiles
QG = 4           # q tiles per DMA group (1MB transfers)
NGRP = QTN // QG
NEG = -60000.0
N_CORES = 8

_compiled = None


def _build():
    import concourse.bass as bass  # noqa: F401
    import concourse.tile as tile
    from concourse import bacc, mybir

    f16, bf16, f32 = mybir.dt.float16, mybir.dt.bfloat16, mybir.dt.float32
    AF = mybir.ActivationFunctionType
    OP = mybir.AluOpType

    nc = bacc.Bacc("TRN2", target_bir_lowering=False, debug=False)

    qTe_d = nc.dram_tensor("qTe", [S, S], f16, kind="ExternalInput")
    kTe_d = nc.dram_tensor("kTe", [S, S], f16, kind="ExternalInput")
    wq_d = nc.dram_tensor("wq", [S, HPC * DK], f16, kind="ExternalInput")
    wk_d = nc.dram_tensor("wk", [S, HPC * DK], f16, kind="ExternalInput")
    bqp_d = nc.dram_tensor("bqp", [128, NPAIR], f32, kind="ExternalInput")
    bkp_d = nc.dram_tensor("bkp", [128, NPAIR], f32, kind="ExternalInput")
    aw2_d = nc.dram_tensor("aw2", [DK, HPC], f16, kind="ExternalInput")
    mb_d = nc.dram_tensor("mb", [S, S], f16, kind="ExternalInput")
    short_d = nc.dram_tensor("shortp", [HPC, S, S], f16, kind="ExternalInput")
    id_d = nc.dram_tensor("ident", [128, 128], f16, kind="ExternalInput")
    bm_d = nc.dram_tensor("bm", [1, 1], f32, kind="ExternalInput")
    out_d = nc.dram_tensor("out", [HPC, S, S], f16, kind="ExternalOutput")

    # partition-major views of the [S, S] planes: [128, 8, 1024]
    mb_v = mb_d[:].rearrange("(n p) d -> p n d", p=128)
    qTe_v = qTe_d[:].rearrange("(n p) d -> p n d", p=128)
    wq_v = wq_d[:].rearrange("(n p) d -> p n d", p=128)
    wk_v = wk_d[:].rearrange("(n p) d -> p n d", p=128)

    with tile.TileContext(nc) as tc, ExitStack() as ctx:
        consts = ctx.enter_context(tc.tile_pool(name="consts", bufs=1))
        stp = ctx.enter_context(tc.tile_pool(name="short_in", bufs=5))
        smp = ctx.enter_context(tc.tile_pool(name="sm", bufs=2))
        ep = ctx.enter_context(tc.tile_pool(name="exp", bufs=3))
        opl = ctx.enter_context(tc.tile_pool(name="outt", bufs=2))
        rsp = ctx.enter_context(tc.tile_pool(name="rows", bufs=8))
        asps = ctx.enter_context(tc.tile_pool(name="asps", bufs=2))
        psp = ctx.enter_context(tc.tile_pool(name="ps", bufs=4, space="PSUM"))

        # ---- PE warmup: trip the HAM busy window while initial DMAs run ----
        wdum = consts.tile([128, 512], f16, tag="wdum")
        nc.vector.memset(wdum[:], 0.0)
        wps = psp.tile([128, 512], f32, tag="ps", name="warm_ps")
        for _ in range(16):
            nc.tensor.matmul(wps[:], wdum[:, 0:128], wdum[:], start=True, stop=True)

        # ---- constant loads: tiny tensors first, then K side for K-proj ----
        aw2_sb = consts.tile([DK, HPC], f16, tag="aw2_sb")
        nc.sync.dma_start(aw2_sb[:], aw2_d[:])
        id_sb = consts.tile([128, 128], f16, tag="id_sb")
        nc.sync.dma_start(id_sb[:], id_d[:])
        bm_sb = consts.tile([1, 1], f32, tag="bm_sb")
        nc.sync.dma_start(bm_sb[:], bm_d[:])
        bqp_sb = consts.tile([128, NPAIR], f32, tag="bqp_sb")
        nc.sync.dma_start(bqp_sb[:], bqp_d[:])
        bkp_sb = consts.tile([128, NPAIR], f32, tag="bkp_sb")
        nc.sync.dma_start(bkp_sb[:], bkp_d[:])
        kt_sb = consts.tile([128, 8, S], f16, tag="kt_sb")
        qt_sb = consts.tile([128, 8, S], f16, tag="qt_sb")
        wk_sb = consts.tile([128, 8, HPC * DK], f16, tag="wk_sb")
        wq_sb = consts.tile([128, 8, HPC * DK], f16, tag="wq_sb")
        nc.sync.dma_start(wk_sb[:], wk_v)
        for kt in range(8):
            nc.sync.dma_start(kt_sb[:, kt, :], kTe_d[kt * 128:(kt + 1) * 128, :])
        nc.sync.dma_start(qt_sb[:], qTe_v)
        nc.sync.dma_start(wq_sb[:], wq_v)
        mb_sb = consts.tile([128, QTN, S], f16, tag="mb_sb")
        nc.sync.dma_start(mb_sb[:], mb_v)

        # per-head transposed projection tiles, 65th row = ones (Q) / aspect (K)
        q65 = [consts.tile([65, S], f16, name=f"q65_{h}", tag=f"q65_{h}")
               for h in range(HPC)]
        k65 = [consts.tile([65, S], f16, name=f"k65_{h}", tag=f"k65_{h}")
               for h in range(HPC)]

        def proj_pieces(w_sb, x_sb, bias_sb, dst65, pr, name, ones_row):
            """Generator: 4 matmul chunks + 1 epilogue, emitted via next()."""
            ps = psp.tile([128, S], f32, tag="ps", name=f"{name}_ps_{pr}")
            for half in range(4):
                for kt in (2 * half, 2 * half + 1):
                    for c in (0, 512):
                        lhsT = w_sb[:, kt, pr * 128:(pr + 1) * 128]
                        rhs = x_sb[:, kt, c:c + 512]
                        nc.tensor.matmul(ps[:, c:c + 512], lhsT, rhs,
                                         start=(kt == 0), stop=(kt == 7))
                yield
            pair = consts.tile([128, S], f16, name=f"pair_{name}_{pr}",
                               tag="pairt", bufs=2)
            # bias add fused into the PSUM->SBUF evacuation
            nc.vector.tensor_scalar(pair[:], ps[:], bias_sb[:, pr:pr + 1],
                                    None, OP.add)
            nc.sync.dma_start(dst65[0][0:64, :], pair[0:64, :])
            nc.sync.dma_start(dst65[1][0:64, :], pair[64:128, :])
            if ones_row:
                nc.vector.memset(dst65[0][64:65, :], 1.0)
                nc.vector.memset(dst65[1][64:65, :], 1.0)
            yield

        def proj(w_sb, x_sb, bias_sb, dst65, pr, name, ones_row=False):
            for _ in proj_pieces(w_sb, x_sb, bias_sb, dst65, pr, name,
                                 ones_row):
                pass

        def aspect_row(h):
            # k65[h][64,:] = tanh(aw_h . k_h + bias_m)
            aps = psp.tile([1, S], f32, tag="ps", name=f"asp_ps_{h}")
            for c in (0, 512):
                nc.tensor.matmul(aps[0:1, c:c + 512], aw2_sb[:, h:h + 1],
                                 k65[h][0:64, c:c + 512], start=True, stop=True)
            ascr = asps.tile([1, S], f16, tag="ascr", name=f"ascr_{h}")
            nc.scalar.activation(ascr[:], aps[0:1, :], AF.Tanh,
                                 bias=bm_sb[0:1, 0:1])
            nc.sync.dma_start(k65[h][64:65, :], ascr[:])

        def kproj(pr):
            proj(wk_sb, kt_sb, bkp_sb, (k65[2 * pr], k65[2 * pr + 1]), pr,
                 "k")

        def kproj_gen(pr):
            return proj_pieces(wk_sb, kt_sb, bkp_sb,
                               (k65[2 * pr], k65[2 * pr + 1]), pr, "k", False)

        def qproj_gen(pr):
            return proj_pieces(wq_sb, qt_sb, bqp_sb,
                               (q65[2 * pr], q65[2 * pr + 1]), pr, "q", True)

        # ---- main loop, interleaved with Q projections so ACT/DVE start
        # as soon as the first head pair is projected ----
        def main_group(h, g, fillers=()):
            short_v = short_d[h].rearrange("(n p) d -> p n d", p=128)
            out_v = out_d[h].rearrange("(n p) d -> p n d", p=128)
            if True:
                q0 = g * QG
                st = stp.tile([128, QG, S], f16, tag="st", name=f"st_{h}_{g}")
                nc.sync.dma_start(st[:], short_v[:, q0:q0 + QG, :])
                sm = smp.tile([128, QG, S], f16, tag="sm", name=f"sm_{h}_{g}")
                nc.vector.tensor_add(sm[:], st[:], mb_sb[:, q0:q0 + QG, :])
                e = ep.tile([128, QG, S], f16, tag="e", name=f"e_{h}_{g}")
                rs = rsp.tile([128, QG], f32, tag="rs", name=f"rs_{h}_{g}")
                rec = rsp.tile([128, QG], f32, tag="rec", name=f"rec_{h}_{g}")
                o = opl.tile([128, QG, S], f16, tag="o", name=f"o_{h}_{g}")
                for j in range(QG):
                    qt = q0 + j
                    ps = psp.tile([128, S], f32, tag="ps", name=f"ps_{h}_{qt}")
                    qsl = q65[h][:, qt * 128:(qt + 1) * 128]
                    nc.tensor.matmul(ps[:, 0:512], qsl, k65[h][:, 0:512],
                                     start=True, stop=False)
                    nc.tensor.matmul(ps[:, 512:1024], qsl, k65[h][:, 512:1024],
                                     start=True, stop=False)
                    nc.tensor.matmul(ps[:, 0:512], id_sb[:], sm[:, j, 0:512],
                                     start=False, stop=True)
                    nc.tensor.matmul(ps[:, 512:1024], id_sb[:], sm[:, j, 512:1024],
                                     start=False, stop=True)
                    nc.scalar.activation(e[:, j, :], ps[:], AF.Exp,
                                         accum_out=rs[:, j:j + 1])
                nc.vector.reciprocal(rec[:], rs[:])
                for j in range(QG):
                    nc.vector.tensor_scalar(o[:, j, :], e[:, j, :],
                                            rec[:, j:j + 1], None, OP.mult)
                if h == HPC - 1 and g == NGRP - 1:
                    nc.sync.dma_start(out_v[:, q0:q0 + 2, :], o[:, 0:2, :])
                    nc.sync.dma_start(out_v[:, q0 + 2:q0 + QG, :], o[:, 2:QG, :])
                else:
                    nc.sync.dma_start(out_v[:, q0:q0 + QG, :], o[:])
                for f in fillers:
                    f()

        def drain(gen, n):
            def f():
                for _ in range(n):
                    try:
                        next(gen)
                    except StopIteration:
                        break
            return f

        # dense prologue: first K pair + its aspect rows + first Q pair
        kproj(0)
        kproj(1)
        for h in range(4):
            aspect_row(h)
        proj(wq_sb, qt_sb, bqp_sb, (q65[0], q65[1]), 0, "q", ones_row=True)

        # remaining projections sprinkled between ACT-paced main groups
        gq1, gk2, gk3, gq2, gq3 = (qproj_gen(1), kproj_gen(2), kproj_gen(3),
                                   qproj_gen(2), qproj_gen(3))
        main_group(0, 0, [drain(gq1, 2)])
        main_group(0, 1, [drain(gq1, 3)])
        main_group(1, 0, [drain(gk2, 2)])
        main_group(1, 1, [drain(gk2, 3)])
        main_group(2, 0, [drain(gk3, 2)])
        main_group(2, 1, [drain(gk3, 3),
                          lambda: aspect_row(4), lambda: aspect_row(5)])
        main_group(3, 0, [drain(gq2, 2)])
        main_group(3, 1, [drain(gq2, 3),
                          lambda: aspect_row(6), lambda: aspect_row(7)])
        main_group(4, 0, [drain(gq3, 2)])
        main_group(4, 1, [drain(gq3, 3)])
        for h in range(5, HPC):
            main_group(h, 0)
            main_group(h, 1)

    nc.compile()
    return nc


def _prep_inputs(query, key, mask, aspect, short, Wq, bq, Wk, bk, Wd, bd,
                 weight_m, bias_m):
    import ml_dtypes
    f16 = np.float16
    bf16 = ml_dtypes.bfloat16
    asp = aspect @ Wd.T + bd                      # [B, DK]
    aw = np.einsum('bc,hcd->bhd', asp, weight_m)  # [B, H, DK]
    ident = np.eye(128, dtype=f16)
    bm = np.asarray(bias_m, np.float32).reshape(1, 1)

    in_maps = []
    for c in range(N_CORES):
        b, g = divmod(c, 2)
        h0 = g * HPC
        sl = slice(h0 * DK, (h0 + HPC) * DK)
        qTe = np.ascontiguousarray(query[b].T).astype(f16)
        kTe = np.ascontiguousarray(key[b].T).astype(f16)
        wq = (Wq[sl].T * 0.125).astype(f16)
        wk = Wk[sl].T.astype(f16)
        bqp = np.ascontiguousarray(
            (bq[sl] * 0.125).astype(np.float32).reshape(NPAIR, 128).T)
        bkp = np.ascontiguousarray(
            bk[sl].astype(np.float32).reshape(NPAIR, 128).T)
        aw2 = np.ascontiguousarray(aw[b, h0:h0 + HPC].T).astype(f16)  # [DK, HPC]
        mb = np.where(mask[b] == 0, np.float32(NEG), np.float32(0)).astype(f16)
        shortp = np.ascontiguousarray(short[b, h0:h0 + HPC]).astype(f16)
        in_maps.append({
            "qTe": qTe, "kTe": kTe, "wq": wq, "wk": wk, "aw2": aw2,
            "mb": mb, "shortp": shortp, "ident": ident, "bm": bm,
            "bqp": bqp, "bkp": bkp,
        })
    return in_maps


def kernel(query, key, mask, aspect, short, Wq, bq, Wk, bk, Wd, bd,
           weight_m, bias_m):
    global _compiled
    from concourse.bass_utils import run_bass_kernel_spmd

    args = [np.asarray(a) for a in (query, key, mask, aspect, short,
                                    Wq, bq, Wk, bk, Wd, bd, weight_m, bias_m)]
    if _compiled is None:
        _compiled = _build()
    nc = _compiled
    in_maps = _prep_inputs(*args)
    res = run_bass_kernel_spmd(nc, in_maps, core_ids=list(range(N_CORES)))
    out = np.empty((B, H, S, S), np.float32)
    for c in range(N_CORES):
        b, g = divmod(c, 2)
        out[b, g * HPC:(g + 1) * HPC] = res.results[c]["out"].astype(np.float32)
    return out



# revision 2
# speedup vs baseline: 1.1377x; 1.1377x over previous
"""Trainium2 Bass kernel for nn_MultiHeadAttention_65481071395029.

8-core SPMD: core c handles batch b=c//2 and heads h0=(c%2)*8 .. h0+8.
Math per core (S=1024, DK=64, 8 heads):
  q = query @ WqT/8 + bq/8        (transposed layout: [dk, s])
  k = key   @ WkT   + bk
  asprow_h = tanh(aw_h . k_h + bias_m)   with aw = (aspect @ WdT + bd) @ weight_m
  scores = q_h.T k_h (+ ones x asprow via 65-row contraction) + (short + maskbias)
  out = softmax(scores, axis=-1) = exp(scores)/rowsum (no max-subtract needed:
  unmasked scores are O(10); masked entries sit at ~-60000 and underflow to 0)

The mask bias is folded into `short` on the host (short + where(mask==0,-6e4,0))
so the device only streams one [128, n, 1024] plane per head and the PE
identity-inject matmuls add it straight from the DMA-landed tile.
All DRAM tensors are host-side pre-permuted to partition-major [128, n, d]
layout so every DMA descriptor is >=8KB contiguous per partition line.
Input loads ride the Sync HWDGE queue (never blocked by compute); output
stores are issued from GpSimd (SWDGE) except the final head, whose stores
use the by-then-idle Sync queue for lower latency.

Engine plan per (head, qtile) iteration over a [128,1024] score tile:
  PE:  2x QK matmuls (fp16, contraction 65 = dk+aspect row) start PSUM,
       2x identity-inject matmuls accumulate short+maskbias on top
  ACT: one Exp pass PSUM->SBUF fp16 with accumulated rowsum
  DVE: reciprocal + 4x tensor_scalar scale
"""

import numpy as np
from contextlib import ExitStack

B, S, D, H, DK = 4, 1024, 1024, 16, 64
HPC = 8          # heads per core
NPAIR = HPC // 2
QTN = S // 128   # q tiles
QG = 4           # q tiles per DMA group (1MB transfers)
NGRP = QTN // QG
NEG = -60000.0
N_CORES = 8

_compiled = None


def _build():
    import concourse.bass as bass  # noqa: F401
    import concourse.tile as tile
    from concourse import bacc, mybir

    f16, f32 = mybir.dt.float16, mybir.dt.float32
    AF = mybir.ActivationFunctionType
    OP = mybir.AluOpType

    nc = bacc.Bacc("TRN2", target_bir_lowering=False, debug=False)

    # all [128, n, d] partition-major, contiguous per partition line
    qt_d = nc.dram_tensor("qt", [128, 8, S], f16, kind="ExternalInput")
    kt_d = nc.dram_tensor("kt", [128, 8, S], f16, kind="ExternalInput")
    wq_d = nc.dram_tensor("wq", [128, 8, HPC * DK], f16, kind="ExternalInput")
    wk_d = nc.dram_tensor("wk", [128, 8, HPC * DK], f16, kind="ExternalInput")
    bqp_d = nc.dram_tensor("bqp", [128, NPAIR], f32, kind="ExternalInput")
    bkp_d = nc.dram_tensor("bkp", [128, NPAIR], f32, kind="ExternalInput")
    aw2_d = nc.dram_tensor("aw2", [DK, HPC], f16, kind="ExternalInput")
    short_d = nc.dram_tensor("shortp", [HPC, 128, QTN, S], f16,
                             kind="ExternalInput")
    id_d = nc.dram_tensor("ident", [128, 128], f16, kind="ExternalInput")
    bm_d = nc.dram_tensor("bm", [1, 1], f32, kind="ExternalInput")
    out_d = nc.dram_tensor("out", [HPC, 128, QTN, S], f16,
                           kind="ExternalOutput")

    with tile.TileContext(nc) as tc, ExitStack() as ctx:
        consts = ctx.enter_context(tc.tile_pool(name="consts", bufs=1))
        stp = ctx.enter_context(tc.tile_pool(name="short_in", bufs=6))
        ep = ctx.enter_context(tc.tile_pool(name="exp", bufs=3))
        opl = ctx.enter_context(tc.tile_pool(name="outt", bufs=2))
        rsp = ctx.enter_context(tc.tile_pool(name="rows", bufs=8))
        asps = ctx.enter_context(tc.tile_pool(name="asps", bufs=2))
        psp = ctx.enter_context(tc.tile_pool(name="ps", bufs=4, space="PSUM"))

        # ---- constant loads: tiny tensors first, then K side for K-proj ----
        aw2_sb = consts.tile([DK, HPC], f16, tag="aw2_sb")
        nc.sync.dma_start(aw2_sb[:], aw2_d[:])
        id_sb = consts.tile([128, 128], f16, tag="id_sb")
        nc.sync.dma_start(id_sb[:], id_d[:])
        bm_sb = consts.tile([1, 1], f32, tag="bm_sb")
        nc.sync.dma_start(bm_sb[:], bm_d[:])
        bqp_sb = consts.tile([128, NPAIR], f32, tag="bqp_sb")
        nc.sync.dma_start(bqp_sb[:], bqp_d[:])
        bkp_sb = consts.tile([128, NPAIR], f32, tag="bkp_sb")
        nc.sync.dma_start(bkp_sb[:], bkp_d[:])
        kt_sb = consts.tile([128, 8, S], f16, tag="kt_sb")
        qt_sb = consts.tile([128, 8, S], f16, tag="qt_sb")
        wk_sb = consts.tile([128, 8, HPC * DK], f16, tag="wk_sb")
        wq_sb = consts.tile([128, 8, HPC * DK], f16, tag="wq_sb")
        nc.sync.dma_start(wk_sb[:], wk_d[:])
        nc.sync.dma_start(kt_sb[:], kt_d[:])
        nc.sync.dma_start(qt_sb[:], qt_d[:])
        nc.sync.dma_start(wq_sb[:], wq_d[:])

        # per-head transposed projection tiles, 65th row = ones (Q) / aspect (K)
        q65 = [consts.tile([65, S], f16, name=f"q65_{h}", tag=f"q65_{h}")
               for h in range(HPC)]
        k65 = [consts.tile([65, S], f16, name=f"k65_{h}", tag=f"k65_{h}")
               for h in range(HPC)]

        def proj_pieces(w_sb, x_sb, bias_sb, dst65, pr, name, ones_row):
            """Generator: 4 matmul chunks + 1 epilogue, emitted via next()."""
            ps = psp.tile([128, S], f32, tag="ps", name=f"{name}_ps_{pr}")
            for half in range(4):
                for kt in (2 * half, 2 * half + 1):
                    for c in (0, 512):
                        lhsT = w_sb[:, kt, pr * 128:(pr + 1) * 128]
                        rhs = x_sb[:, kt, c:c + 512]
                        nc.tensor.matmul(ps[:, c:c + 512], lhsT, rhs,
                                         start=(kt == 0), stop=(kt == 7))
                yield
            pair = consts.tile([128, S], f16, name=f"pair_{name}_{pr}",
                               tag="pairt", bufs=2)
            # bias add fused into the PSUM->SBUF evacuation
            nc.vector.tensor_scalar(pair[:], ps[:], bias_sb[:, pr:pr + 1],
                                    None, OP.add)
            nc.sync.dma_start(dst65[0][0:64, :], pair[0:64, :])
            nc.sync.dma_start(dst65[1][0:64, :], pair[64:128, :])
            if ones_row:
                nc.vector.memset(dst65[0][64:65, :], 1.0)
                nc.vector.memset(dst65[1][64:65, :], 1.0)
            yield

        def proj(w_sb, x_sb, bias_sb, dst65, pr, name, ones_row=False):
            for _ in proj_pieces(w_sb, x_sb, bias_sb, dst65, pr, name,
                                 ones_row):
                pass

        def aspect_row(h):
            # k65[h][64,:] = tanh(aw_h . k_h + bias_m)
            aps = psp.tile([1, S], f32, tag="ps", name=f"asp_ps_{h}")
            for c in (0, 512):
                nc.tensor.matmul(aps[0:1, c:c + 512], aw2_sb[:, h:h + 1],
                                 k65[h][0:64, c:c + 512], start=True, stop=True)
            ascr = asps.tile([1, S], f16, tag="ascr", name=f"ascr_{h}")
            nc.scalar.activation(ascr[:], aps[0:1, :], AF.Tanh,
                                 bias=bm_sb[0:1, 0:1])
            nc.sync.dma_start(k65[h][64:65, :], ascr[:])

        def kproj(pr):
            proj(wk_sb, kt_sb, bkp_sb, (k65[2 * pr], k65[2 * pr + 1]), pr,
                 "k")

        def kproj_gen(pr):
            return proj_pieces(wk_sb, kt_sb, bkp_sb,
                               (k65[2 * pr], k65[2 * pr + 1]), pr, "k", False)

        def qproj_gen(pr):
            return proj_pieces(wq_sb, qt_sb, bqp_sb,
                               (q65[2 * pr], q65[2 * pr + 1]), pr, "q", True)

        # ---- main loop, interleaved with Q projections so ACT/DVE start
        # as soon as the first head pair is projected ----
        def main_group(h, g, fillers=()):
            q0 = g * QG
            st = stp.tile([128, QG, S], f16, tag="st", name=f"st_{h}_{g}")
            nc.sync.dma_start(st[:], short_d[h][:, q0:q0 + QG, :])
            e = ep.tile([128, QG, S], f16, tag="e", name=f"e_{h}_{g}")
            rs = rsp.tile([128, QG], f32, tag="rs", name=f"rs_{h}_{g}")
            rec = rsp.tile([128, QG], f32, tag="rec", name=f"rec_{h}_{g}")
            o = opl.tile([128, QG, S], f16, tag="o", name=f"o_{h}_{g}")
            for j in range(QG):
                qt = q0 + j
                ps = psp.tile([128, S], f32, tag="ps", name=f"ps_{h}_{qt}")
                qsl = q65[h][:, qt * 128:(qt + 1) * 128]
                nc.tensor.matmul(ps[:, 0:512], qsl, k65[h][:, 0:512],
                                 start=True, stop=False)
                nc.tensor.matmul(ps[:, 512:1024], qsl, k65[h][:, 512:1024],
                                 start=True, stop=False)
                nc.tensor.matmul(ps[:, 0:512], id_sb[:], st[:, j, 0:512],
                                 start=False, stop=True)
                nc.tensor.matmul(ps[:, 512:1024], id_sb[:], st[:, j, 512:1024],
                                 start=False, stop=True)
                nc.scalar.activation(e[:, j, :], ps[:], AF.Exp,
                                     accum_out=rs[:, j:j + 1])
            nc.vector.reciprocal(rec[:], rs[:])
            for j in range(QG):
                nc.vector.tensor_scalar(o[:, j, :], e[:, j, :],
                                        rec[:, j:j + 1], None, OP.mult)
            out_v = out_d[h][:, q0:q0 + QG, :]
            if h == HPC - 1:
                # final head: Sync queue is drained of input loads by now;
                # HWDGE has lower completion latency for the kernel tail
                if g == NGRP - 1:
                    nc.sync.dma_start(out_v[:, 0:2, :], o[:, 0:2, :])
                    nc.sync.dma_start(out_v[:, 2:QG, :], o[:, 2:QG, :])
                else:
                    nc.sync.dma_start(out_v, o[:])
            else:
                nc.gpsimd.dma_start(out_v, o[:])
            for f in fillers:
                f()

        def drain(gen, n):
            def f():
                for _ in range(n):
                    try:
                        next(gen)
                    except StopIteration:
                        break
            return f

        # dense prologue: first K pair + its aspect rows + first Q pair
        kproj(0)
        kproj(1)
        for h in range(4):
            aspect_row(h)
        proj(wq_sb, qt_sb, bqp_sb, (q65[0], q65[1]), 0, "q", ones_row=True)

        # remaining projections sprinkled between ACT-paced main groups
        gq1, gk2, gk3, gq2, gq3 = (qproj_gen(1), kproj_gen(2), kproj_gen(3),
                                   qproj_gen(2), qproj_gen(3))
        main_group(0, 0, [drain(gq1, 2)])
        main_group(0, 1, [drain(gq1, 3)])
        main_group(1, 0, [drain(gk2, 2)])
        main_group(1, 1, [drain(gk2, 3)])
        main_group(2, 0, [drain(gk3, 2)])
        main_group(2, 1, [drain(gk3, 3),
                          lambda: aspect_row(4), lambda: aspect_row(5)])
        main_group(3, 0, [drain(gq2, 2)])
        main_group(3, 1, [drain(gq2, 3),
                          lambda: aspect_row(6), lambda: aspect_row(7)])
        main_group(4, 0, [drain(gq3, 2)])
        main_group(4, 1, [drain(gq3, 3)])
        for h in range(5, HPC):
            main_group(h, 0)
            main_group(h, 1)

    nc.compile()
    return nc


def _prep_inputs(query, key, mask, aspect, short, Wq, bq, Wk, bk, Wd, bd,
                 weight_m, bias_m):
    f16 = np.float16
    asp = aspect @ Wd.T + bd                      # [B, DK]
    aw = np.einsum('bc,hcd->bhd', asp, weight_m)  # [B, H, DK]
    ident = np.eye(128, dtype=f16)
    bm = np.asarray(bias_m, np.float32).reshape(1, 1)

    def pmajor(xT):
        # [S, d] -> [128, S//128, d] partition-major, contiguous
        return np.ascontiguousarray(
            xT.reshape(8, 128, xT.shape[1]).transpose(1, 0, 2))

    in_maps = []
    for c in range(N_CORES):
        b, g = divmod(c, 2)
        h0 = g * HPC
        sl = slice(h0 * DK, (h0 + HPC) * DK)
        qt = pmajor(query[b].T.astype(f16))
        kt = pmajor(key[b].T.astype(f16))
        wq = pmajor((Wq[sl].T * 0.125).astype(f16))
        wk = pmajor(Wk[sl].T.astype(f16))
        bqp = np.ascontiguousarray(
            (bq[sl] * 0.125).astype(np.float32).reshape(NPAIR, 128).T)
        bkp = np.ascontiguousarray(
            bk[sl].astype(np.float32).reshape(NPAIR, 128).T)
        aw2 = np.ascontiguousarray(aw[b, h0:h0 + HPC].T).astype(f16)  # [DK, HPC]
        # mask bias folded into short, then partition-major per head
        mbb = np.where(mask[b] == 0, np.float32(NEG), np.float32(0))
        shp = (short[b, h0:h0 + HPC] + mbb[None]).astype(f16)
        shortp = np.ascontiguousarray(
            shp.reshape(HPC, QTN, 128, S).transpose(0, 2, 1, 3))
        in_maps.append({
            "qt": qt, "kt": kt, "wq": wq, "wk": wk, "aw2": aw2,
            "shortp": shortp, "ident": ident, "bm": bm,
            "bqp": bqp, "bkp": bkp,
        })
    return in_maps


def kernel(query, key, mask, aspect, short, Wq, bq, Wk, bk, Wd, bd,
           weight_m, bias_m):
    global _compiled
    from concourse.bass_utils import run_bass_kernel_spmd

    args = [np.asarray(a) for a in (query, key, mask, aspect, short,
                                    Wq, bq, Wk, bk, Wd, bd, weight_m, bias_m)]
    if _compiled is None:
        _compiled = _build()
    nc = _compiled
    in_maps = _prep_inputs(*args)
    res = run_bass_kernel_spmd(nc, in_maps, core_ids=list(range(N_CORES)))
    out = np.empty((B, H, S, S), np.float32)
    for c in range(N_CORES):
        b, g = divmod(c, 2)
        r = res.results[c]["out"]  # [HPC, 128, QTN, S]
        out[b, g * HPC:(g + 1) * HPC] = (
            r.transpose(0, 2, 1, 3).reshape(HPC, S, S).astype(np.float32))
    return out
